# revision 47
# baseline (speedup 1.0000x reference)
"""Trainium2 Bass kernel for nn_LocalAggregator (GNN message passing).

Computes, for hidden (B,N,D) f32, adj (B,HOP,N,N) int64, a (HOP,D) f32:
    e[h,b,i,j] = sum_d a[h,d] * hidden[b,i,d] * hidden[b,j,d]
    e = leaky_relu(e, 0.2)
    tmp[b,i,j] = sum_h exp(e) * (adj[b,h,i,j] == h+1)
    s = rowsum_j(tmp)
    out[b] = (tmp / s) @ hidden[b]

Data-parallel over B across 8 NeuronCores (4 batches per core).

Key structural facts exploited:
  * e_h is SYMMETRIC in (i,j): the e tile computed with j on partitions is
    simultaneously the transposed form tmpT[j,i] needed as the stationary
    operand of the final matmul -- no on-chip transposes at all, provided
    the masks are shipped transposed (host-side layout shuffle).
  * adj holds only values {0,1,2} and is only ever compared against h+1;
    shipping the two comparison planes one-hot-recoded as bf16 {0,1}
    cuts HBM traffic 4x vs int64 and turns the mask step into plain
    bf16 tensor_tensor ops at DVE 2x rate.  hidden ships pre-transposed
    + pre-cast to bf16 (hbT for the e-matmul, hb+ones-column for the
    U-matmul), a^T rides in the last 4 columns of the hbT tile as raw
    f32 bit patterns (bitcast on device).  Output returns bf16.
    Per-core HBM traffic: ~1.8 MiB vs baseline ~5 MiB.
  * The ones column appended to hb makes the U-matmul emit the row sums
    s for free; out = U * (1/s) via DVE reciprocal + tensor_scalar.  The
    U-matmul accumulates the hop sum directly from the two masked-exp
    planes (8 matmuls/batch), removing the DVE hop-add.
  * ACT is the serial driver (4 exps + prelus at 1x, ~1.2GHz).  Batch
    2's prelu runs on DVE as lr = x - 0.8*min(x, 0) (two 1x ops in the
    shadow of the ACT chain); exp runs paired for batches 0/1 and split
    for 2/3 so the trailing mask/U/normalize chain starts as early as
    possible.  Emission order is tuned against the Tile list-scheduler
    (priority = emission order, engines pop the highest-priority READY
    op): the last batch's consumers are emitted ahead of the earlier
    batches' normalize tail, and b0/b1 normalizes ride the idle ACT.

The output leaves in two stores: the b0-b2 bulk as soon as its
normalizes land, then a small b3-only store -- the NRT postamble
semaphore sweep is gated on the LAST store's completion receipt, and a
64KB store's receipt is ~0.4us cheaper than a 256KB one.

Measured on 8xTRN2: ~25.5us median (baseline kernel: 36.5us), rel err
5.7e-3.  Fixed costs dominate what remains: ~6.5us NRT preamble before
the first DMA dispatch, ~1.4us first-load completion receipt, and a
~9.5us postamble (last-store receipt + ~58 fixed NRT semaphore-sweep
ops per engine); the ACT chain (~7.2us) and the b3 dependency tail
(~3.4us) are the remaining algorithmic parts.

The s==0 guard of the reference is dropped: a fully-masked row has
probability ~(4/9)^256 under the randint(0,3) input distribution.
"""

import sys

for _p in ("/opt/trn_rl_repo",):
    if _p not in sys.path:
        sys.path.insert(0, _p)

import numpy as np
import ml_dtypes

import concourse.bacc as bacc
import concourse.mybir as mybir
import concourse.tile as tile
from concourse.bass_utils import run_bass_kernel_spmd

B, N, D, HOP = 32, 256, 128, 2
LRELU_ALPHA = 0.2
NCORES = 8
BLOC = B // NCORES  # batches per core
P = 128  # partitions
NCHUNK = N // P  # 2 chunks of 128 rows
NPAIR = BLOC // 2  # batch pairs for ACT exp fusion

F32 = mybir.dt.float32
BF16 = mybir.dt.bfloat16
AF = mybir.ActivationFunctionType
OP = mybir.AluOpType

BF16NP = np.dtype(ml_dtypes.bfloat16)

T2W = BLOC * N + 4  # 4 bf16 slots holding a^T as f32 bits, then hbT columns

_NC_CACHE = None


def build_nc():
    nc = bacc.Bacc("TRN2", target_bir_lowering=False, debug=False,
                   num_devices=NCORES)

    t2 = nc.dram_tensor("t2", [P, T2W], BF16, kind="ExternalInput")
    hb1 = nc.dram_tensor("hb1", [P, BLOC, NCHUNK, D + 1], BF16,
                         kind="ExternalInput")
    adjm = nc.dram_tensor("adjm", [P, BLOC, NCHUNK, HOP, N], BF16,
                          kind="ExternalInput")
    out = nc.dram_tensor("out", [P, BLOC, NCHUNK, D], BF16,
                         kind="ExternalOutput")

    # raw (non-tile) SBUF tensor for the output staging area so the final
    # store can reference it with a concrete AP after the TileContext
    outs_cm = nc.sbuf_tensor([P, BLOC * NCHUNK * D], BF16)
    outs_h = outs_cm.__enter__()

    with tile.TileContext(nc) as tc:
        with (
            tc.tile_pool(name="const", bufs=1) as constp,
            tc.tile_pool(name="work", bufs=BLOC) as work,
            tc.tile_pool(name="psE", bufs=2, space="PSUM") as psE,
            tc.tile_pool(name="psU", bufs=4, space="PSUM") as psU,
        ):
            # ACT table warm-up: load the Exp/Prelu table set while the
            # input DMAs stream.
            warm_in = constp.tile([P, 1], F32)
            nc.vector.memset(warm_in[:], 0.0)
            warm_out = constp.tile([P, 1], F32)
            nc.scalar.activation(warm_out[:], warm_in[:], AF.Exp)

            # ---- loads (sync HWDGE ring): t2 split per batch so the first
            # e-matmul chain starts as early as possible
            t2s = constp.tile([P, T2W], BF16)
            nc.sync.dma_start(t2s[:, 0:4 + N], t2.ap()[:, 0:4 + N])
            for b in range(1, BLOC):
                nc.sync.dma_start(t2s[:, 4 + b * N:4 + (b + 1) * N],
                                  t2.ap()[:, 4 + b * N:4 + (b + 1) * N])
            adjs = constp.tile([P, BLOC, NCHUNK, HOP, N], BF16)
            nc.sync.dma_start(adjs[:], adjm.ap())
            hb1s = constp.tile([P, BLOC, NCHUNK, D + 1], BF16)
            nc.sync.dma_start(hb1s[:], hb1.ap())
            av = t2s[:, 0:4].bitcast(F32)  # [P, HOP] f32

            # ---- scaled stationaries + e matmuls, per batch:
            #   scT[d, h, b*N+i] = hT[d, b*N+i] * a[h, d]
            #   e_ps[j, jc, h, i] = sum_d hbT[d, jc*128+j] * scT[d, h, i]
            scT = constp.tile([P, HOP, BLOC * N], BF16)
            e_pss = []
            for b in range(BLOC):
                for h in range(HOP):
                    nc.vector.tensor_scalar(
                        scT[:, h, b * N:(b + 1) * N],
                        t2s[:, 4 + b * N:4 + (b + 1) * N], av[:, h:h + 1],
                        None, OP.mult)
                e_ps = psE.tile([P, NCHUNK, HOP, N], F32, tag="e")
                for jc in range(NCHUNK):
                    nc.tensor.matmul(
                        e_ps[:, jc],
                        t2s[:, 4 + b * N + jc * P:4 + b * N + jc * P + P],
                        scT[:, :, b * N:(b + 1) * N],
                        start=True, stop=True)
                e_pss.append(e_ps)

            lr_all = constp.tile([P, BLOC, NCHUNK, HOP, N], BF16)
            ex_all = constp.tile([P, BLOC, NCHUNK, HOP, N], BF16)
            q_all = constp.tile([P, BLOC, NCHUNK, HOP, N], BF16)
            outs = outs_h.ap().rearrange("p (b c d) -> p b c d",
                                         b=BLOC, c=NCHUNK)

            def u_matmul(b):
                # U accumulates the hop sum directly: 8 matmuls over
                # (jc, h) from the masked-exp planes q -- no DVE hop-add.
                u_ps = psU.tile([P, NCHUNK, D + 1], F32, tag="u")
                for ic in range(NCHUNK):
                    for k, (jc, h) in enumerate(
                            (j, h) for j in range(NCHUNK) for h in range(HOP)):
                        nc.tensor.matmul(
                            u_ps[:, ic],
                            q_all[:, b, jc, h, ic * P:(ic + 1) * P],
                            hb1s[:, b, jc, :],
                            start=(k == 0), stop=(k == NCHUNK * HOP - 1))
                rs = work.tile([P, NCHUNK], F32, tag="rs")
                return u_ps, rs

            def recip(u_ps, rs):
                nc.vector.reciprocal(rs[:], u_ps[:, :, D])

            def norm_store(b, u_ps, rs, engine):
                for ic in range(NCHUNK):
                    if engine == "act":
                        nc.scalar.activation(
                            outs[:, b, ic, :], u_ps[:, ic, 0:D], AF.Copy,
                            scale=rs[:, ic:ic + 1])
                    else:
                        nc.vector.tensor_scalar(
                            outs[:, b, ic, :], u_ps[:, ic, 0:D],
                            rs[:, ic:ic + 1], None, OP.mult)

            # ---- pair 0: lrelu per batch, exp/masks fused across the pair.
            # b2's lrelu runs on DVE (2 ops: lr = x - 0.8*min(x,0)) in the
            # shadow of the ACT chain, shortening the serial ACT critical
            # path by one Prelu.
            for b in (0, 1):
                nc.scalar.activation(lr_all[:, b], e_pss[b][:],
                                     AF.Prelu, alpha=LRELU_ALPHA)
            t8 = work.tile([P, NCHUNK, HOP, N], BF16, tag="t8")
            nc.vector.tensor_scalar(t8[:], e_pss[2][:], 0.0, 0.8,
                                    OP.min, OP.mult)
            nc.vector.scalar_tensor_tensor(lr_all[:, 2], e_pss[2][:], 0.0,
                                           t8[:], OP.bypass, OP.subtract)
            nc.scalar.activation(ex_all[:, 0:2], lr_all[:, 0:2], AF.Exp)
            nc.vector.tensor_mul(q_all[:, 0:2], adjs[:, 0:2], ex_all[:, 0:2])
            ur0 = u_matmul(0)
            ur1 = u_matmul(1)

            # ---- pair 1: masks split per batch so U(b2) starts earlier.
            # Emitted BEFORE b0/b1 normalize so the DVE priority queue
            # prefers the critical-path masks over the b0/b1 tail.
            nc.scalar.activation(lr_all[:, 3], e_pss[3][:],
                                 AF.Prelu, alpha=LRELU_ALPHA)
            nc.scalar.activation(ex_all[:, 2], lr_all[:, 2], AF.Exp)
            nc.vector.tensor_mul(q_all[:, 2], adjs[:, 2], ex_all[:, 2])
            # reciprocals for b0/b1 slot here: cheap, and they unblock the
            # ACT-side normalizes without stalling the b3 mask chain below
            recip(*ur0)
            recip(*ur1)
            nc.scalar.activation(ex_all[:, 3], lr_all[:, 3], AF.Exp)
            nc.vector.tensor_mul(q_all[:, 3], adjs[:, 3], ex_all[:, 3])

            # ---- normalize + one fused store; b0/b1 scale on ACT (emitted
            # after the full ACT chain so it cannot displace prelu/exp)
            norm_store(0, *ur0, "act")
            norm_store(1, *ur1, "act")
            u2, rs2 = u_matmul(2)
            recip(u2, rs2)
            norm_store(2, u2, rs2, "dve")
            u3, rs3 = u_matmul(3)
            recip(u3, rs3)
            # split b3's two normalizes across DVE and ACT so they overlap
            nc.vector.tensor_scalar(outs[:, 3, 0, :], u3[:, 0, 0:D],
                                    rs3[:, 0:1], None, OP.mult)
            nc.scalar.activation(outs[:, 3, 1, :], u3[:, 1, 0:D], AF.Copy,
                                 scale=rs3[:, 1:2])
            store_sem = nc.alloc_semaphore("out_store_sem")

    # The output store runs OUTSIDE the TileContext: the tile-exit
    # all-engine barrier otherwise couples every engine's NRT postamble
    # sweep (~6us each, engines are independent) to sync's
    # store-completion wait.  With the store after the barrier, the
    # other engines sweep while sync alone dispatches and waits out the
    # DMA receipt.  The barrier guarantees all `outs` writes are done.
    nc.sync.dma_start(out.ap(), outs[:]).then_inc(store_sem, 16)
    nc.sync.wait_ge(store_sem, 16)
    outs_cm.__exit__(None, None, None)

    nc.compile()
    return nc


def _get_nc():
    global _NC_CACHE
    if _NC_CACHE is None:
        _NC_CACHE = build_nc()
    return _NC_CACHE


def shard_inputs(hidden, adj, a):
    hidden = np.asarray(hidden, dtype=np.float32)
    a = np.asarray(a, dtype=np.float32)
    adj = np.asarray(adj)

    # t2: [128, 4 + B*N] per core: a^T (f32) as raw bit patterns in 4
    # bf16 slots, then hidden^T batches side by side
    ht = np.ascontiguousarray(hidden.transpose(2, 0, 1))  # (D, B, N)
    a_bits = np.ascontiguousarray(a.T.astype(np.float32)).view(np.uint16)

    # hb1: [128, B, NCHUNK, D+1] with ones column
    hb = hidden.reshape(B, NCHUNK, P, D).transpose(2, 0, 1, 3)  # (P,B,jc,D)
    hb1_full = np.empty((P, B, NCHUNK, D + 1), dtype=BF16NP)
    hb1_full[..., :D] = hb.astype(BF16NP)
    hb1_full[..., D] = 1.0

    # adjm: one-hot mask planes, transposed: [128, B, NCHUNK, HOP, N]
    #   adjm[p, b, jc, h, i] = (adj[b, h, i, jc*128+p] == h+1)
    targets = np.arange(1, HOP + 1, dtype=adj.dtype)[None, :, None, None, None]
    m = (adj.reshape(B, HOP, N, NCHUNK, P) == targets)
    adjm_full = np.ascontiguousarray(
        m.transpose(4, 0, 3, 1, 2)).astype(BF16NP)  # (P, B, jc, HOP, N)

    in_maps = []
    for c in range(NCORES):
        lo, hi = c * BLOC, (c + 1) * BLOC
        t2c = np.empty((P, T2W), dtype=BF16NP)
        t2c[:, 4:] = ht[:, lo:hi, :].reshape(P, BLOC * N).astype(BF16NP)
        t2c.view(np.uint16)[:, 0:4] = a_bits
        in_maps.append({
            "t2": t2c,
            "hb1": np.ascontiguousarray(hb1_full[:, lo:hi]),
            "adjm": np.ascontiguousarray(adjm_full[:, lo:hi]),
        })
    return in_maps


def run(hidden, adj, a, trace=False):
    nc = _get_nc()
    in_maps = shard_inputs(hidden, adj, a)
    res = run_bass_kernel_spmd(nc, in_maps, list(range(NCORES)), trace=trace)
    # out per core: (BLOC, P, NCHUNK, D) bf16 -> (BLOC, N, D) f32
    parts = []
    for i in range(NCORES):
        o = np.asarray(res.results[i]["out"])  # (P, BLOC, NCHUNK, D)
        parts.append(o.transpose(1, 2, 0, 3).reshape(BLOC, N, D))
    return np.concatenate(parts, axis=0).astype(np.float32), res


def kernel(hidden, adj, a):
    return run(hidden, adj, a)[0]


# revision 48
# speedup vs baseline: 1.0375x; 1.0375x over previous
"""Trainium2 Bass kernel for nn_LocalAggregator (GNN message passing).

Computes, for hidden (B,N,D) f32, adj (B,HOP,N,N) int64, a (HOP,D) f32:
    e[h,b,i,j] = sum_d a[h,d] * hidden[b,i,d] * hidden[b,j,d]
    e = leaky_relu(e, 0.2)
    tmp[b,i,j] = sum_h exp(e) * (adj[b,h,i,j] == h+1)
    s = rowsum_j(tmp)
    out[b] = (tmp / s) @ hidden[b]

Data-parallel over B across 8 NeuronCores (4 batches per core).

Key structural facts exploited:
  * e_h is SYMMETRIC in (i,j): the e tile computed with j on partitions is
    simultaneously the transposed form tmpT[j,i] needed as the stationary
    operand of the final matmul -- no on-chip transposes at all, provided
    the masks are shipped transposed (host-side layout shuffle).
  * adj holds only values {0,1,2} and is only ever compared against h+1;
    shipping the two comparison planes one-hot-recoded as bf16 {0,1}
    cuts HBM traffic 4x vs int64 and turns the mask step into plain
    bf16 tensor_tensor ops at DVE 2x rate.  hidden ships pre-transposed
    + pre-cast to bf16 (hbT for the e-matmul, hb+ones-column for the
    U-matmul), a^T rides in the last 4 columns of the hbT tile as raw
    f32 bit patterns (bitcast on device).  Output returns bf16.
    Per-core HBM traffic: ~1.8 MiB vs baseline ~5 MiB.
  * The ones column appended to hb makes the U-matmul emit the row sums
    s for free; out = U * (1/s) via DVE reciprocal + tensor_scalar.  The
    U-matmul accumulates the hop sum directly from the two masked-exp
    planes (8 matmuls/batch), removing the DVE hop-add.
  * ACT is the serial driver (4 exps + prelus at 1x, ~1.2GHz).  Batch
    2's prelu runs on DVE as lr = x - 0.8*min(x, 0) (two 1x ops in the
    shadow of the ACT chain); exp runs paired for batches 0/1 and split
    for 2/3 so the trailing mask/U/normalize chain starts as early as
    possible.  Emission order is tuned against the Tile list-scheduler
    (priority = emission order, engines pop the highest-priority READY
    op): the last batch's consumers are emitted ahead of the earlier
    batches' normalize tail, and b0/b1 normalizes ride the idle ACT.

The output leaves in two stores: the b0-b2 bulk as soon as its
normalizes land, then a small b3-only store -- the NRT postamble
semaphore sweep is gated on the LAST store's completion receipt, and a
64KB store's receipt is ~0.4us cheaper than a 256KB one.

Measured on 8xTRN2: ~25.5us median (baseline kernel: 36.5us), rel err
5.7e-3.  Fixed costs dominate what remains: ~6.5us NRT preamble before
the first DMA dispatch, ~1.4us first-load completion receipt, and a
~9.5us postamble (last-store receipt + ~58 fixed NRT semaphore-sweep
ops per engine); the ACT chain (~7.2us) and the b3 dependency tail
(~3.4us) are the remaining algorithmic parts.

The s==0 guard of the reference is dropped: a fully-masked row has
probability ~(4/9)^256 under the randint(0,3) input distribution.
"""

import sys

for _p in ("/opt/trn_rl_repo",):
    if _p not in sys.path:
        sys.path.insert(0, _p)

import numpy as np
import ml_dtypes

import concourse.bacc as bacc
import concourse.mybir as mybir
import concourse.tile as tile
from concourse.bass_utils import run_bass_kernel_spmd

B, N, D, HOP = 32, 256, 128, 2
LRELU_ALPHA = 0.2
NCORES = 8
BLOC = B // NCORES  # batches per core
P = 128  # partitions
NCHUNK = N // P  # 2 chunks of 128 rows
NPAIR = BLOC // 2  # batch pairs for ACT exp fusion

F32 = mybir.dt.float32
BF16 = mybir.dt.bfloat16
AF = mybir.ActivationFunctionType
OP = mybir.AluOpType

BF16NP = np.dtype(ml_dtypes.bfloat16)

T2W = BLOC * N + 4  # 4 bf16 slots holding a^T as f32 bits, then hbT columns

_NC_CACHE = None


def build_nc():
    nc = bacc.Bacc("TRN2", target_bir_lowering=False, debug=False,
                   num_devices=NCORES)

    t2 = nc.dram_tensor("t2", [P, T2W], BF16, kind="ExternalInput")
    hb1 = nc.dram_tensor("hb1", [P, BLOC, NCHUNK, D + 1], BF16,
                         kind="ExternalInput")
    adjm = nc.dram_tensor("adjm", [P, BLOC, NCHUNK, HOP, N], BF16,
                          kind="ExternalInput")
    out = nc.dram_tensor("out", [P, BLOC, NCHUNK, D], BF16,
                         kind="ExternalOutput")

    with tile.TileContext(nc) as tc:
        with (
            tc.tile_pool(name="const", bufs=1) as constp,
            tc.tile_pool(name="work", bufs=BLOC) as work,
            tc.tile_pool(name="psE", bufs=2, space="PSUM") as psE,
            tc.tile_pool(name="psU", bufs=4, space="PSUM") as psU,
        ):
            # ACT table warm-up: load the Exp/Prelu table set while the
            # input DMAs stream.
            warm_in = constp.tile([P, 1], F32)
            nc.vector.memset(warm_in[:], 0.0)
            warm_out = constp.tile([P, 1], F32)
            nc.scalar.activation(warm_out[:], warm_in[:], AF.Exp)

            # ---- loads (sync HWDGE ring): t2 split per batch so the first
            # e-matmul chain starts as early as possible
            t2s = constp.tile([P, T2W], BF16)
            nc.sync.dma_start(t2s[:, 0:4 + N], t2.ap()[:, 0:4 + N])
            for b in range(1, BLOC):
                nc.sync.dma_start(t2s[:, 4 + b * N:4 + (b + 1) * N],
                                  t2.ap()[:, 4 + b * N:4 + (b + 1) * N])
            adjs = constp.tile([P, BLOC, NCHUNK, HOP, N], BF16)
            nc.sync.dma_start(adjs[:], adjm.ap())
            hb1s = constp.tile([P, BLOC, NCHUNK, D + 1], BF16)
            nc.sync.dma_start(hb1s[:], hb1.ap())
            av = t2s[:, 0:4].bitcast(F32)  # [P, HOP] f32

            # ---- scaled stationaries + e matmuls, per batch:
            #   scT[d, h, b*N+i] = hT[d, b*N+i] * a[h, d]
            #   e_ps[j, jc, h, i] = sum_d hbT[d, jc*128+j] * scT[d, h, i]
            scT = constp.tile([P, HOP, BLOC * N], BF16)
            e_pss = []
            for b in range(BLOC):
                for h in range(HOP):
                    nc.vector.tensor_scalar(
                        scT[:, h, b * N:(b + 1) * N],
                        t2s[:, 4 + b * N:4 + (b + 1) * N], av[:, h:h + 1],
                        None, OP.mult)
                e_ps = psE.tile([P, NCHUNK, HOP, N], F32, tag="e")
                for jc in range(NCHUNK):
                    nc.tensor.matmul(
                        e_ps[:, jc],
                        t2s[:, 4 + b * N + jc * P:4 + b * N + jc * P + P],
                        scT[:, :, b * N:(b + 1) * N],
                        start=True, stop=True)
                e_pss.append(e_ps)

            lr_all = constp.tile([P, BLOC, NCHUNK, HOP, N], BF16)
            ex_all = constp.tile([P, BLOC, NCHUNK, HOP, N], BF16)
            q_all = constp.tile([P, BLOC, NCHUNK, HOP, N], BF16)
            outs = constp.tile([P, BLOC, NCHUNK, D], BF16)

            def u_matmul(b):
                # U accumulates the hop sum directly: 8 matmuls over
                # (jc, h) from the masked-exp planes q -- no DVE hop-add.
                u_ps = psU.tile([P, NCHUNK, D + 1], F32, tag="u")
                for ic in range(NCHUNK):
                    for k, (jc, h) in enumerate(
                            (j, h) for j in range(NCHUNK) for h in range(HOP)):
                        nc.tensor.matmul(
                            u_ps[:, ic],
                            q_all[:, b, jc, h, ic * P:(ic + 1) * P],
                            hb1s[:, b, jc, :],
                            start=(k == 0), stop=(k == NCHUNK * HOP - 1))
                rs = work.tile([P, NCHUNK], F32, tag="rs")
                return u_ps, rs

            def recip(u_ps, rs):
                nc.vector.reciprocal(rs[:], u_ps[:, :, D])

            def norm_store(b, u_ps, rs, engine):
                for ic in range(NCHUNK):
                    if engine == "act":
                        nc.scalar.activation(
                            outs[:, b, ic, :], u_ps[:, ic, 0:D], AF.Copy,
                            scale=rs[:, ic:ic + 1])
                    else:
                        nc.vector.tensor_scalar(
                            outs[:, b, ic, :], u_ps[:, ic, 0:D],
                            rs[:, ic:ic + 1], None, OP.mult)

            # ---- pair 0: lrelu per batch, exp/masks fused across the pair.
            # b2's lrelu runs on DVE (2 ops: lr = x - 0.8*min(x,0)) in the
            # shadow of the ACT chain, shortening the serial ACT critical
            # path by one Prelu.
            for b in (0, 1):
                nc.scalar.activation(lr_all[:, b], e_pss[b][:],
                                     AF.Prelu, alpha=LRELU_ALPHA)
            t8 = work.tile([P, NCHUNK, HOP, N], BF16, tag="t8")
            nc.vector.tensor_scalar(t8[:], e_pss[2][:], 0.0, 0.8,
                                    OP.min, OP.mult)
            nc.vector.scalar_tensor_tensor(lr_all[:, 2], e_pss[2][:], 0.0,
                                           t8[:], OP.bypass, OP.subtract)
            nc.scalar.activation(ex_all[:, 0:2], lr_all[:, 0:2], AF.Exp)
            nc.vector.tensor_mul(q_all[:, 0:2], adjs[:, 0:2], ex_all[:, 0:2])
            ur0 = u_matmul(0)
            ur1 = u_matmul(1)

            # ---- pair 1: masks split per batch so U(b2) starts earlier.
            # Emitted BEFORE b0/b1 normalize so the DVE priority queue
            # prefers the critical-path masks over the b0/b1 tail.
            nc.scalar.activation(lr_all[:, 3], e_pss[3][:],
                                 AF.Prelu, alpha=LRELU_ALPHA)
            nc.scalar.activation(ex_all[:, 2], lr_all[:, 2], AF.Exp)
            nc.vector.tensor_mul(q_all[:, 2], adjs[:, 2], ex_all[:, 2])
            # reciprocals for b0/b1 slot here: cheap, and they unblock the
            # ACT-side normalizes without stalling the b3 mask chain below
            recip(*ur0)
            recip(*ur1)
            nc.scalar.activation(ex_all[:, 3], lr_all[:, 3], AF.Exp)
            nc.vector.tensor_mul(q_all[:, 3], adjs[:, 3], ex_all[:, 3])

            # ---- normalize + one fused store; b0/b1 scale on ACT (emitted
            # after the full ACT chain so it cannot displace prelu/exp)
            norm_store(0, *ur0, "act")
            norm_store(1, *ur1, "act")
            u2, rs2 = u_matmul(2)
            recip(u2, rs2)
            norm_store(2, u2, rs2, "dve")
            u3, rs3 = u_matmul(3)
            recip(u3, rs3)
            # split b3's two normalizes across DVE and ACT so they overlap
            nc.vector.tensor_scalar(outs[:, 3, 0, :], u3[:, 0, 0:D],
                                    rs3[:, 0:1], None, OP.mult)
            nc.scalar.activation(outs[:, 3, 1, :], u3[:, 1, 0:D], AF.Copy,
                                 scale=rs3[:, 1:2])
            # two stores: bulk (b0-b2) as soon as its norms land, then a
            # small b3-only store -- the NRT postamble sweep gates on the
            # LAST store's completion receipt, so keep it tiny and early
            nc.sync.dma_start(out.ap()[:, 0:3], outs[:, 0:3])
            nc.sync.dma_start(out.ap()[:, 3], outs[:, 3])

    nc.compile()
    return nc


def _get_nc():
    global _NC_CACHE
    if _NC_CACHE is None:
        _NC_CACHE = build_nc()
    return _NC_CACHE


def shard_inputs(hidden, adj, a):
    hidden = np.asarray(hidden, dtype=np.float32)
    a = np.asarray(a, dtype=np.float32)
    adj = np.asarray(adj)

    # t2: [128, 4 + B*N] per core: a^T (f32) as raw bit patterns in 4
    # bf16 slots, then hidden^T batches side by side
    ht = np.ascontiguousarray(hidden.transpose(2, 0, 1))  # (D, B, N)
    a_bits = np.ascontiguousarray(a.T.astype(np.float32)).view(np.uint16)

    # hb1: [128, B, NCHUNK, D+1] with ones column
    hb = hidden.reshape(B, NCHUNK, P, D).transpose(2, 0, 1, 3)  # (P,B,jc,D)
    hb1_full = np.empty((P, B, NCHUNK, D + 1), dtype=BF16NP)
    hb1_full[..., :D] = hb.astype(BF16NP)
    hb1_full[..., D] = 1.0

    # adjm: one-hot mask planes, transposed: [128, B, NCHUNK, HOP, N]
    #   adjm[p, b, jc, h, i] = (adj[b, h, i, jc*128+p] == h+1)
    targets = np.arange(1, HOP + 1, dtype=adj.dtype)[None, :, None, None, None]
    m = (adj.reshape(B, HOP, N, NCHUNK, P) == targets)
    adjm_full = np.ascontiguousarray(
        m.transpose(4, 0, 3, 1, 2)).astype(BF16NP)  # (P, B, jc, HOP, N)

    in_maps = []
    for c in range(NCORES):
        lo, hi = c * BLOC, (c + 1) * BLOC
        t2c = np.empty((P, T2W), dtype=BF16NP)
        t2c[:, 4:] = ht[:, lo:hi, :].reshape(P, BLOC * N).astype(BF16NP)
        t2c.view(np.uint16)[:, 0:4] = a_bits
        in_maps.append({
            "t2": t2c,
            "hb1": np.ascontiguousarray(hb1_full[:, lo:hi]),
            "adjm": np.ascontiguousarray(adjm_full[:, lo:hi]),
        })
    return in_maps


def run(hidden, adj, a, trace=False):
    nc = _get_nc()
    in_maps = shard_inputs(hidden, adj, a)
    res = run_bass_kernel_spmd(nc, in_maps, list(range(NCORES)), trace=trace)
    # out per core: (BLOC, P, NCHUNK, D) bf16 -> (BLOC, N, D) f32
    parts = []
    for i in range(NCORES):
        o = np.asarray(res.results[i]["out"])  # (P, BLOC, NCHUNK, D)
        parts.append(o.transpose(1, 2, 0, 3).reshape(BLOC, N, D))
    return np.concatenate(parts, axis=0).astype(np.float32), res


def kernel(hidden, adj, a):
    return run(hidden, adj, a)[0]
